# revision 1
# baseline (speedup 1.0000x reference)
"""Trainium2 Bass kernel for nn_MoE_16664473108485 (moe_routing).

Computation (reference):
    concat = features.transpose(1,0,2).reshape(B, E*D)      # [B, 1024]
    h      = gelu(concat @ gate_w1 + gate_b1)               # [B, 128]
    h      = layernorm(h) * ln1_g + ln1_b
    logits = h @ gate_w2 + gate_b2                          # [B, 8]
    scores = softmax(logits)
    out[e] = layernorm(scores[:, e, None] * features[e]) * out_g + out_b

Strategy: pure data-parallel over B across 8 cores.  Inside each core,
process 128-sample tiles:
  - one DMA brings [128, E*D] f32 features
  - PE transposes each expert block to get X^T (contraction over D needs
    D on partitions), PSUM->SBUF copy casts to bf16
  - 8 accumulating bf16 matmuls compute the gate hidden layer
  - gelu on ScalarE (reads PSUM), LN1 stats via bn_stats/bn_aggr
  - h_ln transposed once on PE, logits matmul, Exp with fused row-sum
  - final per-expert LayerNorm(score*x) folded to x*A - mean(x)*A with
        A = z * sqrt(D) * rsqrt(z^2*M2 + D*eps*Z^2)
    (z = exp(logit), Z = sum_e z, M2 = sum_d (x - mean)^2), so no
    softmax division is ever materialized; per-(sample,expert) stats come
    from one grouped bn_stats over [128, 8, 128].
"""

import numpy as np
from contextlib import ExitStack

E = 8
D = 128
H = 128
P = 128           # samples per tile (partition dim)
GROUP = 8         # b-tiles per batching group for the small ops
EPS = 1e-5
HALF_LN_D = 0.5 * float(np.log(128.0))
N_CORES = 8

_NC_CACHE = {}


def _build_nc(b_loc, has_b1, has_ln1, has_b2, has_outgb, num_devices=1,
              repeats=1):
    import concourse.bass as bass
    import concourse.tile as tile
    from concourse import bacc, mybir, masks

    f32 = mybir.dt.float32
    bf16 = mybir.dt.bfloat16
    AO = mybir.AluOpType
    AF = mybir.ActivationFunctionType

    assert b_loc % (P * GROUP) == 0
    n_groups = b_loc // (P * GROUP)

    nc = bacc.Bacc(
        "TRN2",
        target_bir_lowering=False,
        debug=False,
        enable_asserts=False,
        num_devices=num_devices,
    )

    feat = nc.dram_tensor("features", [E, b_loc, D], f32, kind="ExternalInput").ap()
    w1 = nc.dram_tensor("w1bf", [E, D, H], bf16, kind="ExternalInput").ap()
    w2 = nc.dram_tensor("w2bf", [H, E], bf16, kind="ExternalInput").ap()
    out = nc.dram_tensor("out", [E, b_loc, D], f32, kind="ExternalOutput").ap()
    if has_b1:
        b1row = nc.dram_tensor("b1row", [1, H], bf16, kind="ExternalInput").ap()
    if has_ln1:
        g_ln1 = nc.dram_tensor("g_ln1", [P, H], f32, kind="ExternalInput").ap()
        b_ln1 = nc.dram_tensor("b_ln1", [P, H], f32, kind="ExternalInput").ap()
    if has_b2:
        eb2 = nc.dram_tensor("eb2", [P, E], f32, kind="ExternalInput").ap()
    if has_outgb:
        g_out = nc.dram_tensor("g_out", [P, D], f32, kind="ExternalInput").ap()
        b_out = nc.dram_tensor("b_out", [P, D], f32, kind="ExternalInput").ap()

    feat_t = feat.rearrange("e (n p) d -> n p e d", p=P)
    out_t = out.rearrange("e (n p) d -> n p e d", p=P)

    with tile.TileContext(nc) as tc, ExitStack() as ctx:
        # Chain every table-function ACT op in emission order so the Tile
        # scheduler cannot interleave ops from different act-function sets
        # (each set switch costs a ~1.3us LoadActFuncSet).
        _act_prev = [None]

        def act_ordered(inst):
            ins = inst.ins
            if _act_prev[0] is not None:
                tile.add_dep_helper(ins, _act_prev[0], sync=False,
                                    reason="act-table order")
            _act_prev[0] = ins
            return inst

        def act_load(set_id):
            # Pre-place the activation-table load; set 10 = gelu+helpers,
            # set 6 = ln+exp+helpers.  Without this, the compiler picks a
            # separate table per function and thrashes ~1.3us reloads.
            return act_ordered(nc.scalar.add_instruction(
                mybir.InstLoadActFuncSet(
                    name=nc.get_next_instruction_name(), ins=[], outs=[],
                    act_func_set_id=set_id)))

        const_pool = ctx.enter_context(tc.tile_pool(name="const", bufs=1))
        ident_f = const_pool.tile([P, P], f32)
        masks.make_identity(nc, ident_f[:])
        ident_b = const_pool.tile([P, P], bf16)
        masks.make_identity(nc, ident_b[:])
        w1_sb = const_pool.tile([D, E * H], bf16)
        w1_3 = w1_sb.rearrange("d (e h) -> d e h", e=E)
        nc.sync.dma_start(w1_3, w1.rearrange("e d h -> d e h"))
        w2_sb = const_pool.tile([H, E], bf16)
        nc.sync.dma_start(w2_sb[:], w2)
        if has_b1:
            ones1 = const_pool.tile([1, P], bf16)
            nc.vector.memset(ones1[:], 1.0)
            b1_sb = const_pool.tile([1, H], bf16)
            nc.sync.dma_start(b1_sb[:], b1row)
        if has_ln1:
            gln_sb = const_pool.tile([P, H], f32)
            nc.sync.dma_start(gln_sb[:], g_ln1)
            bln_sb = const_pool.tile([P, H], f32)
            nc.sync.dma_start(bln_sb[:], b_ln1)
        if has_b2:
            eb2_sb = const_pool.tile([P, E], f32)
            nc.sync.dma_start(eb2_sb[:], eb2)
        if has_outgb:
            gout_sb = const_pool.tile([P, D], f32)
            nc.sync.dma_start(gout_sb[:], g_out)
            bout_sb = const_pool.tile([P, D], f32)
            nc.sync.dma_start(bout_sb[:], b_out)

        ones_d = const_pool.tile([D, 1], bf16)
        nc.vector.memset(ones_d[:], 1.0)
        hld = const_pool.tile([P, 1], f32)
        nc.vector.memset(hld[:], HALF_LN_D)

        io_pool = ctx.enter_context(tc.tile_pool(name="io", bufs=GROUP + 4))
        xt_pool = ctx.enter_context(tc.tile_pool(name="xt", bufs=3))
        osb_pool = ctx.enter_context(tc.tile_pool(name="osb", bufs=3))
        hg_pool = ctx.enter_context(tc.tile_pool(name="hg", bufs=GROUP + 4))
        sm_pool = ctx.enter_context(tc.tile_pool(name="sm", bufs=3))
        sq_pool = ctx.enter_context(tc.tile_pool(name="sq", bufs=2))
        grp_pool = ctx.enter_context(tc.tile_pool(name="grp", bufs=2))
        ps_t = ctx.enter_context(tc.tile_pool(name="ps_t", bufs=2, space="PSUM"))
        ps_h = ctx.enter_context(tc.tile_pool(name="ps_h", bufs=2, space="PSUM"))
        ps_lg = ctx.enter_context(tc.tile_pool(name="ps_lg", bufs=2, space="PSUM"))
        ps_m = ctx.enter_context(tc.tile_pool(name="ps_m", bufs=2, space="PSUM"))

        if repeats > 1:
            # timing-only variant: repeat the whole body in a HW loop so a
            # single dispatch carries R x the steady-state work
            rep_ctx = tc.For_i(0, repeats, 1)
            rep_ctx.__enter__()

        for g in range(n_groups):
            # ---- group-level stat tiles ----
            # per-(sample, tile-in-group, expert) sum(x) accumulates in PSUM
            pm = ps_m.tile([P, GROUP * E], f32, tag="pm", name=f"pm_{g}")
            # per-(sample, tile-in-group, expert) sum(x^2), from GPSIMD
            sqs = grp_pool.tile([P, GROUP * E], f32, tag="sqs")
            sqs3 = sqs.rearrange("p (j e) -> p j e", j=GROUP)
            ln_mv = grp_pool.tile([P, GROUP * 2], f32, tag="ln_mv")
            ln3 = ln_mv.rearrange("p (j s) -> p j s", j=GROUP)
            zz = grp_pool.tile([P, GROUP * E], f32, tag="zz")
            zz3 = zz.rearrange("p (j e) -> p j e", j=GROUP)
            zs = grp_pool.tile([P, GROUP], f32, tag="zs")

            xfs = []
            hgs = []
            act_load(10)
            # ---- phase 1: per-tile gate pipeline up to LN1 stats ----
            for j in range(GROUP):
                i = g * GROUP + j
                xf = io_pool.tile([P, E * D], f32, tag="xf", name=f"xf_{i}")
                xf3 = xf.rearrange("p (e d) -> p e d", e=E)
                nc.sync.dma_start(xf3, feat_t[i])
                xfs.append(xf)

                # per-(sample, expert) sum(x^2): square on GPSIMD (idle
                # engine), then one grouped reduce on DVE
                sqscr = sq_pool.tile([P, E * D], f32, tag="sqscr", name=f"sqscr_{i}")
                nc.gpsimd.tensor_mul(sqscr[:], xf[:], xf[:])
                nc.vector.reduce_sum(
                    sqs3[:, j], sqscr.rearrange("p (e d) -> p e d", e=E),
                    axis=mybir.AxisListType.X,
                )

                # transpose each expert block: [b, d] -> [d, b] (2 waves of 4)
                xt = xt_pool.tile([P, E * D], bf16, tag="xt", name=f"xt_{i}")
                xt3 = xt.rearrange("p (e b) -> p e b", e=E)
                for w in range(2):
                    pst = ps_t.tile([P, 512], f32, tag="pst", name=f"pst_{i}_{w}")
                    for k in range(4):
                        e = w * 4 + k
                        nc.tensor.matmul(
                            pst[:, k * P:(k + 1) * P], xf3[:, e], ident_f[:],
                            is_transpose=True,
                        )
                    nc.any.tensor_copy(xt[:, w * 512:(w + 1) * 512], pst[:])

                # gate hidden: accumulate over experts into PSUM [b, h];
                # also per-expert sum(x) via a ones-column stream
                ph = ps_h.tile([P, H], f32, tag="ph", name=f"ph_{i}")
                for e in range(E):
                    nc.tensor.matmul(
                        ph[:], xt3[:, e], w1_3[:, e],
                        start=(e == 0), stop=(e == E - 1 and not has_b1),
                    )
                    nc.tensor.matmul(
                        pm[:, j * E + e:j * E + e + 1], xt3[:, e], ones_d[:],
                        start=True, stop=True,
                    )
                if has_b1:
                    nc.tensor.matmul(ph[:], ones1[:], b1_sb[:], start=False, stop=True)

                hg = hg_pool.tile([P, H], f32, tag="hg", name=f"hg_{i}")
                act_ordered(nc.scalar.activation(hg[:], ph[:], AF.Gelu,
                                                 bias=0.0, scale=1.0))
                hgs.append(hg)

                s1 = sm_pool.tile([P, 6], f32, tag="s1", name=f"s1_{i}")
                nc.vector.bn_stats(s1[:], hg[:])
                nc.vector.bn_aggr(ln3[:, j], s1[:])

            # ---- phase 2: batched LN1 scalar math ----
            act_load(6)
            veps = grp_pool.tile([P, GROUP], f32, tag="veps")
            nc.vector.tensor_scalar(veps[:], ln3[:, :, 1], EPS, None, AO.add)
            lnv = grp_pool.tile([P, GROUP], f32, tag="lnv")
            act_ordered(nc.scalar.activation(lnv[:], veps[:], AF.Ln,
                                             bias=0.0, scale=1.0))
            rs1 = grp_pool.tile([P, GROUP], f32, tag="rs1")
            act_ordered(nc.scalar.activation(rs1[:], lnv[:], AF.Exp,
                                             bias=0.0, scale=-0.5))
            mb1 = grp_pool.tile([P, GROUP], f32, tag="mb1")
            nc.vector.tensor_mul(mb1[:], ln3[:, :, 0], rs1[:])

            # ---- phase 3: per-tile LN1 apply -> logits -> exp ----
            for j in range(GROUP):
                i = g * GROUP + j
                hln = sm_pool.tile([P, H], f32, tag="hln", name=f"hln_{i}")
                nc.vector.tensor_scalar(
                    hln[:], hgs[j][:], rs1[:, j:j + 1], mb1[:, j:j + 1],
                    AO.mult, AO.subtract,
                )
                if has_ln1:
                    nc.vector.tensor_mul(hln[:], hln[:], gln_sb[:])
                    nc.vector.tensor_add(hln[:], hln[:], bln_sb[:])

                plg = ps_lg.tile([P, 512], f32, tag="plg", name=f"plg_{i}")
                nc.tensor.matmul(plg[:, 0:P], hln[:], ident_f[:], is_transpose=True)
                hlt = sm_pool.tile([P, P], bf16, tag="hlt", name=f"hlt_{i}")
                nc.any.tensor_copy(hlt[:], plg[:, 0:P])
                nc.tensor.matmul(plg[:, P:P + E], hlt[:], w2_sb[:],
                                 start=True, stop=True)

                if has_b2:
                    act_ordered(nc.scalar.activation(zz3[:, j], plg[:, P:P + E],
                                                     AF.Exp, bias=0.0, scale=1.0))
                    nc.vector.tensor_mul(zz3[:, j], zz3[:, j], eb2_sb[:])
                    nc.vector.reduce_sum(zs[:, j:j + 1], zz3[:, j],
                                         axis=mybir.AxisListType.X)
                else:
                    act_ordered(nc.scalar.activation(zz3[:, j], plg[:, P:P + E],
                                                     AF.Exp, bias=0.0, scale=1.0,
                                                     accum_out=zs[:, j:j + 1]))

            # ---- phase 4: batched final-LN scalar math ----
            # m2 = sum(x^2) - sum(x)^2/D  (musum from PSUM, sumsq from GPSIMD)
            ms = grp_pool.tile([P, GROUP * E], f32, tag="ms")
            ms3 = ms.rearrange("p (j e) -> p j e", j=GROUP)
            nc.any.tensor_copy(ms[:], pm[:])
            msq = grp_pool.tile([P, GROUP * E], f32, tag="msq")
            nc.vector.scalar_tensor_tensor(msq[:], ms[:], 1.0 / D, ms[:],
                                           AO.mult, AO.mult)
            m2 = grp_pool.tile([P, GROUP * E], f32, tag="m2")
            m23 = m2.rearrange("p (j e) -> p j e", j=GROUP)
            nc.vector.tensor_sub(m2[:], sqs[:], msq[:])

            # u = z^2 * m2 ; u2 = u + (D*eps)*Z^2 ; A = z*sqrt(D)*rsqrt(u2)
            tt = grp_pool.tile([P, GROUP * E], f32, tag="tt")
            tt3 = tt.rearrange("p (j e) -> p j e", j=GROUP)
            nc.vector.tensor_mul(tt3[:], zz3[:], zz3[:])
            uu = grp_pool.tile([P, GROUP * E], f32, tag="uu")
            uu3 = uu.rearrange("p (j e) -> p j e", j=GROUP)
            nc.vector.tensor_mul(uu3[:], tt3[:], m23[:])
            zeps = grp_pool.tile([P, GROUP], f32, tag="zeps")
            nc.vector.scalar_tensor_tensor(zeps[:], zs[:], float(D) * EPS, zs[:],
                                           AO.mult, AO.mult)
            u2 = grp_pool.tile([P, GROUP * E], f32, tag="u2")
            u23 = u2.rearrange("p (j e) -> p j e", j=GROUP)
            for j in range(GROUP):
                nc.vector.tensor_scalar(u23[:, j], uu3[:, j], zeps[:, j:j + 1],
                                        None, AO.add)
            l2 = grp_pool.tile([P, GROUP * E], f32, tag="l2")
            act_ordered(nc.scalar.activation(l2[:], u2[:], AF.Ln,
                                             bias=0.0, scale=1.0))
            qq = grp_pool.tile([P, GROUP * E], f32, tag="qq")
            # exp(-0.5*ln(u2) + 0.5*ln(D)) = sqrt(D) * rsqrt(u2)
            act_ordered(nc.scalar.activation(qq[:], l2[:], AF.Exp,
                                             bias=hld[:], scale=-0.5))
            aa = grp_pool.tile([P, GROUP * E], f32, tag="aa")
            aa3 = aa.rearrange("p (j e) -> p j e", j=GROUP)
            nc.vector.tensor_mul(aa3[:], zz3[:], qq.rearrange("p (j e) -> p j e", j=GROUP)[:])
            # B = mean * A = (musum / D) * A
            bb = grp_pool.tile([P, GROUP * E], f32, tag="bb")
            bb3 = bb.rearrange("p (j e) -> p j e", j=GROUP)
            nc.vector.scalar_tensor_tensor(bb3[:], ms3[:], 1.0 / D, aa3[:],
                                           AO.mult, AO.mult)

            # ---- phase 5: per-tile final applies + store ----
            for j in range(GROUP):
                i = g * GROUP + j
                xf3 = xfs[j].rearrange("p (e d) -> p e d", e=E)
                osb = osb_pool.tile([P, E * D], f32, tag="osb", name=f"osb_{i}")
                osb3 = osb.rearrange("p (e d) -> p e d", e=E)
                for e in range(E):
                    eng = (nc.vector, nc.scalar, nc.gpsimd)[e % 3] if False else None
                    nc.any.tensor_scalar(
                        osb3[:, e], xf3[:, e],
                        aa3[:, j, e:e + 1], bb3[:, j, e:e + 1],
                        AO.mult, AO.subtract,
                    )
                    if has_outgb:
                        nc.vector.tensor_mul(osb3[:, e], osb3[:, e], gout_sb[:])
                        nc.vector.tensor_add(osb3[:, e], osb3[:, e], bout_sb[:])
                nc.scalar.dma_start(out_t[i], osb3)

        if repeats > 1:
            rep_ctx.__exit__(None, None, None)

    nc.compile()
    return nc


def _get_nc(b_loc, flags, num_devices):
    key = (b_loc, flags, num_devices)
    if key not in _NC_CACHE:
        _NC_CACHE[key] = _build_nc(b_loc, *flags, num_devices=num_devices)
    return _NC_CACHE[key]


def kernel(**inputs):
    import ml_dtypes
    from concourse.bass_utils import run_bass_kernel_spmd

    features = np.asarray(inputs["features"], dtype=np.float32)
    gate_w1 = np.asarray(inputs["gate_w1"], dtype=np.float32)
    gate_b1 = np.asarray(inputs["gate_b1"], dtype=np.float32)
    ln1_g = np.asarray(inputs["ln1_g"], dtype=np.float32)
    ln1_b = np.asarray(inputs["ln1_b"], dtype=np.float32)
    gate_w2 = np.asarray(inputs["gate_w2"], dtype=np.float32)
    gate_b2 = np.asarray(inputs["gate_b2"], dtype=np.float32)
    out_g = np.asarray(inputs["out_g"], dtype=np.float32)
    out_b = np.asarray(inputs["out_b"], dtype=np.float32)

    e, B, d = features.shape
    assert e == E and d == D
    assert B % (N_CORES * P * GROUP) == 0
    b_loc = B // N_CORES

    has_b1 = bool(np.any(gate_b1 != 0))
    has_ln1 = bool(np.any(ln1_g != 1) or np.any(ln1_b != 0))
    has_b2 = bool(np.any(gate_b2 != 0))
    has_outgb = bool(np.any(out_g != 1) or np.any(out_b != 0))
    flags = (has_b1, has_ln1, has_b2, has_outgb)

    nc = _get_nc(b_loc, flags, num_devices=1)

    bf = ml_dtypes.bfloat16
    w1bf = np.ascontiguousarray(gate_w1.reshape(E, D, H).astype(bf))
    w2bf = np.ascontiguousarray(gate_w2.astype(bf))

    common = {"w1bf": w1bf, "w2bf": w2bf}
    if has_b1:
        common["b1row"] = np.ascontiguousarray(gate_b1.reshape(1, H).astype(bf))
    if has_ln1:
        common["g_ln1"] = np.ascontiguousarray(np.tile(ln1_g, (P, 1)))
        common["b_ln1"] = np.ascontiguousarray(np.tile(ln1_b, (P, 1)))
    if has_b2:
        common["eb2"] = np.ascontiguousarray(
            np.tile(np.exp(gate_b2.astype(np.float64)).astype(np.float32), (P, 1)))
    if has_outgb:
        common["g_out"] = np.ascontiguousarray(np.tile(out_g, (P, 1)))
        common["b_out"] = np.ascontiguousarray(np.tile(out_b, (P, 1)))

    in_maps = []
    for c in range(N_CORES):
        m = dict(common)
        m["features"] = np.ascontiguousarray(
            features[:, c * b_loc:(c + 1) * b_loc, :])
        in_maps.append(m)

    res = run_bass_kernel_spmd(nc, in_maps, core_ids=list(range(N_CORES)))
    global LAST_RESULTS
    LAST_RESULTS = res
    out = np.concatenate([r["out"] for r in res.results], axis=1)
    return np.ascontiguousarray(out, dtype=np.float32)


LAST_RESULTS = None



# revision 6
# speedup vs baseline: 1.5263x; 1.5263x over previous
"""Trainium2 Bass kernel for nn_MoE_16664473108485 (moe_routing).

Computation (reference):
    concat = features.transpose(1,0,2).reshape(B, E*D)      # [B, 1024]
    h      = gelu(concat @ gate_w1 + gate_b1)               # [B, 128]
    h      = layernorm(h) * ln1_g + ln1_b
    logits = h @ gate_w2 + gate_b2                          # [B, 8]
    scores = softmax(logits)
    out[e] = layernorm(scores[:, e, None] * features[e]) * out_g + out_b

v2 strategy (pure data-parallel over B across 8 cores):
  - features are converted to bf16 on the host; output is written bf16 and
    upconverted on the host (rel-err budget is 2e-2; bf16 adds ~1.5e-3).
    This halves HBM traffic both ways.
  - Per block of 2048 samples, one fully-linear 4 MiB DMA per direction:
    partition p holds samples 16p..16p+15 (JJ=16 samples per partition),
    giving 4 KiB contiguous per partition line per expert.
  - Per 128-sample sub-tile jj: PE transposes each expert block to bf16
    PSUM, a PSUM->SBUF copy feeds the accumulating gate matmuls.  The gate
    rhs is [w1_e | delta-ones block], so per-expert sums sum_d(x) fall out
    of the same accumulation for free (no extra LDWEIGHTS).
  - sum_d(x^2): GPSIMD squares the transposed tiles (batched, to amortize
    the ~1us GPSIMD fixed cost), and PE ones-matmuls reduce over d (which
    is the partition dim in the transposed layout), so DVE never touches
    the O(B*E*D) reduction.
  - Final per-expert LayerNorm(score*x) folded to x*A + Bn with
        A = z*sqrt(D)*rsqrt(z^2*M2 + D*eps*Z^2),  Bn = -(s/D)*A
    (z = exp(logit), Z = sum_e z, s = sum_d x, M2 = sum x^2 - s^2/D), so no
    softmax division is ever materialized.
  - Scalar-engine table thrash avoided by batching all Gelu ops of a block
    before the Ln/Exp ops (2 table loads per 2048 samples).
"""

import numpy as np
from contextlib import ExitStack

E = 8
D = 128
H = 128
P = 128           # partitions
JJ = 16           # samples per partition per block
BLK = P * JJ      # 2048 samples per block
CW = H + 8        # gate rhs width: w1 columns + delta-ones block
EPS = 1e-5
HALF_LN_D = 0.5 * float(np.log(128.0))
N_CORES = 8

_NC_CACHE = {}


def _build_nc(b_loc, has_b1, has_ln1, has_b2, has_outgb, num_devices=1,
              sim_tanh=False):
    import concourse.bass as bass
    import concourse.tile as tile
    from concourse import bacc, mybir, masks

    f32 = mybir.dt.float32
    bf16 = mybir.dt.bfloat16
    AO = mybir.AluOpType
    AF = mybir.ActivationFunctionType

    assert b_loc % BLK == 0
    n_blocks = b_loc // BLK

    nc = bacc.Bacc(
        "TRN2",
        target_bir_lowering=False,
        debug=False,
        enable_asserts=False,
        num_devices=num_devices,
    )

    featb = nc.dram_tensor("featb", [E, b_loc, D], bf16, kind="ExternalInput").ap()
    w1x = nc.dram_tensor("w1x", [D, E * CW], bf16, kind="ExternalInput").ap()
    qones = nc.dram_tensor("qones", [D, E * 8], bf16, kind="ExternalInput").ap()
    w2 = nc.dram_tensor("w2bf", [H, E], bf16, kind="ExternalInput").ap()
    outb = nc.dram_tensor("outb", [E, b_loc, D], bf16, kind="ExternalOutput").ap()
    if has_b1:
        b1row = nc.dram_tensor("b1row", [1, H], bf16, kind="ExternalInput").ap()
    if has_ln1:
        g_ln1 = nc.dram_tensor("g_ln1", [P, H], f32, kind="ExternalInput").ap()
        b_ln1 = nc.dram_tensor("b_ln1", [P, H], f32, kind="ExternalInput").ap()
    if has_b2:
        eb2 = nc.dram_tensor("eb2", [P, E], f32, kind="ExternalInput").ap()
    if has_outgb:
        g_out = nc.dram_tensor("g_out", [P, D], f32, kind="ExternalInput").ap()
        b_out = nc.dram_tensor("b_out", [P, D], f32, kind="ExternalInput").ap()

    feat_r = featb.rearrange("e (n p jj) d -> n p e jj d", p=P, jj=JJ)
    out_r = outb.rearrange("e (n p jj) d -> n p e jj d", p=P, jj=JJ)

    with tile.TileContext(nc) as tc, ExitStack() as ctx:
        # Chain every table-function ACT op in emission order so the Tile
        # scheduler cannot interleave ops from different act-function sets
        # (each set switch costs a ~1.3us LoadActFuncSet).
        _act_prev = [None]

        def act_ordered(inst):
            ins = inst.ins
            if _act_prev[0] is not None:
                tile.add_dep_helper(ins, _act_prev[0], sync=False,
                                    reason="act-table order")
            _act_prev[0] = ins
            return inst

        def act_load(set_id):
            # set 10 = gelu+helpers, set 6 = ln+exp+helpers
            return act_ordered(nc.scalar.add_instruction(
                mybir.InstLoadActFuncSet(
                    name=nc.get_next_instruction_name(), ins=[], outs=[],
                    act_func_set_id=set_id)))

        const_pool = ctx.enter_context(tc.tile_pool(name="const", bufs=1))
        ident_b = const_pool.tile([P, P], bf16)
        masks.make_identity(nc, ident_b[:])
        w1x_sb = const_pool.tile([D, E * CW], bf16)
        nc.sync.dma_start(w1x_sb[:], w1x)
        w1x3 = w1x_sb.rearrange("d (e c) -> d e c", e=E)
        qo_sb = const_pool.tile([D, E * 8], bf16)
        nc.sync.dma_start(qo_sb[:], qones)
        qo3 = qo_sb.rearrange("d (e c) -> d e c", e=E)
        w2_sb = const_pool.tile([H, E], bf16)
        nc.sync.dma_start(w2_sb[:], w2)
        hld = const_pool.tile([P, 1], f32)
        nc.vector.memset(hld[:], HALF_LN_D)
        if has_b1:
            ones1 = const_pool.tile([1, P], bf16)
            nc.vector.memset(ones1[:], 1.0)
            b1_sb = const_pool.tile([1, H], bf16)
            nc.sync.dma_start(b1_sb[:], b1row)
        if has_ln1:
            gln_sb = const_pool.tile([P, H], f32)
            nc.sync.dma_start(gln_sb[:], g_ln1)
            bln_sb = const_pool.tile([P, H], f32)
            nc.sync.dma_start(bln_sb[:], b_ln1)
        if has_b2:
            eb2_sb = const_pool.tile([P, E], f32)
            nc.sync.dma_start(eb2_sb[:], eb2)
        if has_outgb:
            gout_sb = const_pool.tile([P, D], f32)
            nc.sync.dma_start(gout_sb[:], g_out)
            bout_sb = const_pool.tile([P, D], f32)
            nc.sync.dma_start(bout_sb[:], b_out)

        io_pool = ctx.enter_context(tc.tile_pool(name="io", bufs=3))
        xt_pool = ctx.enter_context(tc.tile_pool(name="xt", bufs=2))
        hb_pool = ctx.enter_context(tc.tile_pool(name="hb", bufs=2))
        sm_pool = ctx.enter_context(tc.tile_pool(name="sm", bufs=4))
        st_pool = ctx.enter_context(tc.tile_pool(name="st", bufs=2))
        ps_t = ctx.enter_context(tc.tile_pool(name="ps_t", bufs=2, space="PSUM"))
        ps_g = ctx.enter_context(tc.tile_pool(name="ps_g", bufs=2, space="PSUM"))
        ps_s = ctx.enter_context(tc.tile_pool(name="ps_s", bufs=2, space="PSUM"))
        ps_h = ctx.enter_context(tc.tile_pool(name="ps_h", bufs=2, space="PSUM"))

        SQ_BATCH = 4  # jj sub-tiles per gpsimd square op

        for n in range(n_blocks):
            x = io_pool.tile([P, E * JJ * D], bf16, tag="x", name=f"x_{n}")
            x4 = x.rearrange("p (e jj d) -> p e jj d", e=E, jj=JJ)
            nc.sync.dma_start(x4, feat_r[n])

            # block-level stat tiles
            sS = st_pool.tile([P, JJ * E], f32, tag="sS")     # sum_d x
            sS3 = sS.rearrange("p (jj e) -> p jj e", jj=JJ)
            sQ = st_pool.tile([P, JJ * E], f32, tag="sQ")     # sum_d x^2
            zz = st_pool.tile([P, JJ * E], f32, tag="zz")     # exp(logits)
            zz3 = zz.rearrange("p (jj e) -> p jj e", jj=JJ)
            zs = st_pool.tile([P, JJ], f32, tag="zs")         # Z = sum_e z
            ln = st_pool.tile([P, JJ * 2], f32, tag="ln")     # LN1 mean/var
            ln3 = ln.rearrange("p (jj s) -> p jj s", jj=JJ)

            hg = hb_pool.tile([P, JJ * H], bf16, tag="hg", name=f"hg_{n}")
            hg3 = hg.rearrange("p (jj h) -> p jj h", jj=JJ)
            hl = hb_pool.tile([P, JJ * H], bf16, tag="hl", name=f"hl_{n}")
            hl3 = hl.rearrange("p (jj h) -> p jj h", jj=JJ)

            # per-block PSUM stats bank: q sums (cols 0:128) + logits (128:256)
            psS = ps_s.tile([P, 512], f32, tag="psS", name=f"psS_{n}")

            act_load(10)
            xts = []
            # ---- phase 1: transposes, gate matmuls, squares, LN1 stats ----
            for jj in range(JJ):
                psT = ps_t.tile([P, E * D], bf16, tag="psT", name=f"psT_{n}_{jj}")
                psT3 = psT.rearrange("p (e b) -> p e b", e=E)
                for e in range(E):
                    nc.tensor.matmul(psT3[:, e], x4[:, e, jj], ident_b[:],
                                     is_transpose=True)

                if jj % SQ_BATCH == 0:
                    xtg = xt_pool.tile([P, SQ_BATCH * E * D], bf16, tag="xtg",
                                       name=f"xtg_{n}_{jj // SQ_BATCH}")
                    xqg = xt_pool.tile([P, SQ_BATCH * E * D], bf16, tag="xqg",
                                       name=f"xqg_{n}_{jj // SQ_BATCH}")
                    xts.append((xtg, xqg))
                k = jj % SQ_BATCH
                xt3 = xtg.rearrange("p (k e b) -> p k e b", k=SQ_BATCH, e=E)[:, k]
                # PSUM -> SBUF copy of the transposed tile (split DVE/ACT)
                if jj % 2 == 0:
                    act_ordered(nc.scalar.activation(
                        xtg.rearrange("p (k r) -> p k r", k=SQ_BATCH)[:, k],
                        psT[:], AF.Copy))
                else:
                    nc.vector.tensor_copy(
                        xtg.rearrange("p (k r) -> p k r", k=SQ_BATCH)[:, k],
                        psT[:])

                # gate hidden accumulation; rhs [w1_e | delta-ones] also
                # produces per-expert sums in cols H..H+7
                psG = ps_g.tile([P, 512], f32, tag="psG", name=f"psG_{n}_{jj}")
                for e in range(E):
                    nc.tensor.matmul(
                        psG[:, 0:CW], xt3[:, e], w1x3[:, e],
                        start=(e == 0), stop=(e == E - 1 and not has_b1),
                    )
                if has_b1:
                    nc.tensor.matmul(psG[:, 0:H], ones1[:], b1_sb[:],
                                     start=False, stop=True)

                if k == SQ_BATCH - 1:
                    # square the transposed tiles (batched for GPSIMD), then
                    # sum_d x^2 via PE ones-matmuls (d = partition dim here)
                    nc.gpsimd.tensor_mul(xqg[:], xtg[:], xtg[:])
                    xqg4 = xqg.rearrange("p (k e b) -> p k e b", k=SQ_BATCH, e=E)
                    for kq in range(SQ_BATCH):
                        jq = jj - (SQ_BATCH - 1) + kq
                        for e in range(E):
                            nc.tensor.matmul(
                                psS[:, jq * E:(jq + 1) * E], xqg4[:, kq, e],
                                qo3[:, e],
                                start=(e == 0), stop=(e == E - 1),
                            )

                gelu_fn = AF.Tanh if sim_tanh else AF.Gelu
                act_ordered(nc.scalar.activation(hg3[:, jj], psG[:, 0:H],
                                                 gelu_fn, bias=0.0, scale=1.0))
                s1 = sm_pool.tile([P, 6], f32, tag="s1", name=f"s1_{n}_{jj}")
                nc.vector.bn_stats(s1[:], hg3[:, jj])
                nc.vector.bn_aggr(ln3[:, jj], s1[:])
                # copy per-expert sums out of the gate bank
                nc.vector.tensor_copy(sS3[:, jj], psG[:, H:H + E])

            # ---- phase 2: batched LN1 scalar math ----
            act_load(6)
            veps = st_pool.tile([P, JJ], f32, tag="veps")
            nc.vector.tensor_scalar(veps[:], ln3[:, :, 1], EPS, None, AO.add)
            lnv = st_pool.tile([P, JJ], f32, tag="lnv")
            act_ordered(nc.scalar.activation(lnv[:], veps[:], AF.Ln,
                                             bias=0.0, scale=1.0))
            rs1 = st_pool.tile([P, JJ], f32, tag="rs1")
            act_ordered(nc.scalar.activation(rs1[:], lnv[:], AF.Exp,
                                             bias=0.0, scale=-0.5))
            mb1n = st_pool.tile([P, JJ], f32, tag="mb1n")
            nc.vector.scalar_tensor_tensor(mb1n[:], ln3[:, :, 0], -1.0, rs1[:],
                                           AO.mult, AO.mult)

            # ---- phase 3: LN1 apply -> logits -> exp ----
            for jj in range(JJ):
                nc.vector.tensor_scalar(
                    hl3[:, jj], hg3[:, jj], rs1[:, jj:jj + 1],
                    mb1n[:, jj:jj + 1], AO.mult, AO.add,
                )
                if has_ln1:
                    nc.vector.tensor_mul(hl3[:, jj], hl3[:, jj], gln_sb[:])
                    nc.vector.tensor_add(hl3[:, jj], hl3[:, jj], bln_sb[:])
                psH = ps_h.tile([P, H], bf16, tag="psH", name=f"psH_{n}_{jj}")
                nc.tensor.matmul(psH[:], hl3[:, jj], ident_b[:], is_transpose=True)
                hlt = sm_pool.tile([P, H], bf16, tag="hlt", name=f"hlt_{n}_{jj}")
                nc.vector.tensor_copy(hlt[:], psH[:])
                nc.tensor.matmul(psS[:, 256 + jj * E:256 + (jj + 1) * E],
                                 hlt[:], w2_sb[:], start=True, stop=True)
                act_ordered(nc.scalar.activation(
                    zz3[:, jj], psS[:, 256 + jj * E:256 + (jj + 1) * E],
                    AF.Exp, bias=0.0, scale=1.0))
                if has_b2:
                    nc.vector.tensor_mul(zz3[:, jj], zz3[:, jj], eb2_sb[:])

            # ---- phase 4: batched final-LN scalar math ----
            nc.vector.reduce_sum(zs[:], zz3, axis=mybir.AxisListType.X)
            nc.vector.tensor_copy(sQ[:], psS[:, 0:JJ * E])
            msq = st_pool.tile([P, JJ * E], f32, tag="msq")
            nc.vector.scalar_tensor_tensor(msq[:], sS[:], 1.0 / D, sS[:],
                                           AO.mult, AO.mult)
            m2 = st_pool.tile([P, JJ * E], f32, tag="m2")
            nc.vector.tensor_sub(m2[:], sQ[:], msq[:])
            zz2 = st_pool.tile([P, JJ * E], f32, tag="zz2")
            nc.vector.tensor_mul(zz2[:], zz[:], zz[:])
            u = st_pool.tile([P, JJ * E], f32, tag="u")
            nc.vector.tensor_mul(u[:], zz2[:], m2[:])
            zeps = st_pool.tile([P, JJ], f32, tag="zeps")
            nc.vector.scalar_tensor_tensor(zeps[:], zs[:], float(D) * EPS, zs[:],
                                           AO.mult, AO.mult)
            u2 = st_pool.tile([P, JJ * E], f32, tag="u2")
            u23 = u2.rearrange("p (jj e) -> p jj e", jj=JJ)
            u3 = u.rearrange("p (jj e) -> p jj e", jj=JJ)
            for jj in range(JJ):
                nc.vector.tensor_scalar(u23[:, jj], u3[:, jj],
                                        zeps[:, jj:jj + 1], None, AO.add)
            l2 = st_pool.tile([P, JJ * E], f32, tag="l2")
            act_ordered(nc.scalar.activation(l2[:], u2[:], AF.Ln,
                                             bias=0.0, scale=1.0))
            qq = st_pool.tile([P, JJ * E], f32, tag="qq")
            # exp(-0.5*ln(u2) + 0.5*ln(D)) = sqrt(D) * rsqrt(u2)
            act_ordered(nc.scalar.activation(qq[:], l2[:], AF.Exp,
                                             bias=hld[:], scale=-0.5))
            aa = st_pool.tile([P, JJ * E], f32, tag="aa")
            nc.vector.tensor_mul(aa[:], zz[:], qq[:])
            bn = st_pool.tile([P, JJ * E], f32, tag="bn")
            nc.vector.scalar_tensor_tensor(bn[:], sS[:], -1.0 / D, aa[:],
                                           AO.mult, AO.mult)

            # ---- phase 5: final applies (in place) + store ----
            for jj in range(JJ):
                for e in range(E):
                    c = jj * E + e
                    args = (x4[:, e, jj], x4[:, e, jj],
                            aa[:, c:c + 1], bn[:, c:c + 1], AO.mult, AO.add)
                    if c % 4 == 3:
                        act_ordered(nc.scalar.activation(
                            x4[:, e, jj], x4[:, e, jj], AF.Identity,
                            bias=bn[:, c:c + 1], scale=aa[:, c:c + 1]))
                    else:
                        nc.vector.tensor_scalar(*args)
                    if has_outgb:
                        nc.vector.tensor_mul(x4[:, e, jj], x4[:, e, jj],
                                             gout_sb[:])
                        nc.vector.tensor_add(x4[:, e, jj], x4[:, e, jj],
                                             bout_sb[:])
            nc.scalar.dma_start(out_r[n], x4)

    nc.compile()
    return nc


def _get_nc(b_loc, flags, num_devices):
    key = (b_loc, flags, num_devices)
    if key not in _NC_CACHE:
        _NC_CACHE[key] = _build_nc(b_loc, *flags, num_devices=num_devices)
    return _NC_CACHE[key]


def _host_inputs(gate_w1, gate_b1, ln1_g, ln1_b, gate_w2, gate_b2, out_g, out_b,
                 flags):
    import ml_dtypes
    bf = ml_dtypes.bfloat16
    has_b1, has_ln1, has_b2, has_outgb = flags

    w1r = gate_w1.reshape(E, D, H)
    w1x = np.zeros((D, E, CW), dtype=bf)
    w1x[:, :, 0:H] = w1r.transpose(1, 0, 2).astype(bf)
    for e in range(E):
        w1x[:, e, H + e] = bf(1.0)
    qones = np.zeros((D, E, 8), dtype=bf)
    for e in range(E):
        qones[:, e, e] = bf(1.0)

    common = {
        "w1x": np.ascontiguousarray(w1x.reshape(D, E * CW)),
        "qones": np.ascontiguousarray(qones.reshape(D, E * 8)),
        "w2bf": np.ascontiguousarray(gate_w2.astype(bf)),
    }
    if has_b1:
        common["b1row"] = np.ascontiguousarray(gate_b1.reshape(1, H).astype(bf))
    if has_ln1:
        common["g_ln1"] = np.ascontiguousarray(np.tile(ln1_g, (P, 1)))
        common["b_ln1"] = np.ascontiguousarray(np.tile(ln1_b, (P, 1)))
    if has_b2:
        common["eb2"] = np.ascontiguousarray(
            np.tile(np.exp(gate_b2.astype(np.float64)).astype(np.float32),
                    (P, 1)))
    if has_outgb:
        common["g_out"] = np.ascontiguousarray(np.tile(out_g, (P, 1)))
        common["b_out"] = np.ascontiguousarray(np.tile(out_b, (P, 1)))
    return common


def kernel(**inputs):
    import ml_dtypes
    from concourse.bass_utils import run_bass_kernel_spmd

    features = np.asarray(inputs["features"], dtype=np.float32)
    gate_w1 = np.asarray(inputs["gate_w1"], dtype=np.float32)
    gate_b1 = np.asarray(inputs["gate_b1"], dtype=np.float32)
    ln1_g = np.asarray(inputs["ln1_g"], dtype=np.float32)
    ln1_b = np.asarray(inputs["ln1_b"], dtype=np.float32)
    gate_w2 = np.asarray(inputs["gate_w2"], dtype=np.float32)
    gate_b2 = np.asarray(inputs["gate_b2"], dtype=np.float32)
    out_g = np.asarray(inputs["out_g"], dtype=np.float32)
    out_b = np.asarray(inputs["out_b"], dtype=np.float32)

    e, B, d = features.shape
    assert e == E and d == D
    assert B % (N_CORES * BLK) == 0
    b_loc = B // N_CORES

    has_b1 = bool(np.any(gate_b1 != 0))
    has_ln1 = bool(np.any(ln1_g != 1) or np.any(ln1_b != 0))
    has_b2 = bool(np.any(gate_b2 != 0))
    has_outgb = bool(np.any(out_g != 1) or np.any(out_b != 0))
    flags = (has_b1, has_ln1, has_b2, has_outgb)

    nc = _get_nc(b_loc, flags, num_devices=1)

    bf = ml_dtypes.bfloat16
    common = _host_inputs(gate_w1, gate_b1, ln1_g, ln1_b, gate_w2, gate_b2,
                          out_g, out_b, flags)
    featb = features.astype(bf)

    in_maps = []
    for c in range(N_CORES):
        m = dict(common)
        m["featb"] = np.ascontiguousarray(featb[:, c * b_loc:(c + 1) * b_loc, :])
        in_maps.append(m)

    res = run_bass_kernel_spmd(nc, in_maps, core_ids=list(range(N_CORES)))
    global LAST_RESULTS
    LAST_RESULTS = res
    out = np.concatenate([r["outb"] for r in res.results], axis=1)
    return np.ascontiguousarray(out.astype(np.float32))


LAST_RESULTS = None


# revision 12
# speedup vs baseline: 1.5909x; 1.0423x over previous
"""Trainium2 Bass kernel for nn_MoE_16664473108485 (moe_routing).

Computation (reference):
    concat = features.transpose(1,0,2).reshape(B, E*D)      # [B, 1024]
    h      = gelu(concat @ gate_w1 + gate_b1)               # [B, 128]
    h      = layernorm(h) * ln1_g + ln1_b
    logits = h @ gate_w2 + gate_b2                          # [B, 8]
    scores = softmax(logits)
    out[e] = layernorm(scores[:, e, None] * features[e]) * out_g + out_b

v2 strategy (pure data-parallel over B across 8 cores):
  - features are converted to bf16 on the host; output is written bf16 and
    upconverted on the host (rel-err budget is 2e-2; bf16 adds ~1.5e-3).
    This halves HBM traffic both ways.
  - Per block of 2048 samples, one fully-linear 4 MiB DMA per direction:
    partition p holds samples 16p..16p+15 (JJ=16 samples per partition),
    giving 4 KiB contiguous per partition line per expert.
  - Per 128-sample sub-tile jj: PE transposes each expert block to bf16
    PSUM, a PSUM->SBUF copy feeds the accumulating gate matmuls.  The gate
    rhs is [w1_e | delta-ones block], so per-expert sums sum_d(x) fall out
    of the same accumulation for free (no extra LDWEIGHTS).
  - sum_d(x^2): GPSIMD squares the transposed tiles (batched, to amortize
    the ~1us GPSIMD fixed cost), and PE ones-matmuls reduce over d (which
    is the partition dim in the transposed layout), so DVE never touches
    the O(B*E*D) reduction.
  - Final per-expert LayerNorm(score*x) folded to x*A + Bn with
        A = z*sqrt(D)*rsqrt(z^2*M2 + D*eps*Z^2),  Bn = -(s/D)*A
    (z = exp(logit), Z = sum_e z, s = sum_d x, M2 = sum x^2 - s^2/D), so no
    softmax division is ever materialized.
  - Scalar-engine table thrash avoided by batching all Gelu ops of a block
    before the Ln/Exp ops (2 table loads per 2048 samples).
"""

import numpy as np
from contextlib import ExitStack

E = 8
D = 128
H = 128
P = 128           # partitions
JJ = 16           # samples per partition per block
BLK = P * JJ      # 2048 samples per block
CW = H + 8        # gate rhs width: w1 columns + delta-ones block
EPS = 1e-5
HALF_LN_D = 0.5 * float(np.log(128.0))
N_CORES = 8

_NC_CACHE = {}


def _build_nc(b_loc, has_b1, has_ln1, has_b2, has_outgb, num_devices=1,
              sim_tanh=False):
    import concourse.bass as bass
    import concourse.tile as tile
    from concourse import bacc, mybir, masks

    f32 = mybir.dt.float32
    bf16 = mybir.dt.bfloat16
    AO = mybir.AluOpType
    AF = mybir.ActivationFunctionType

    assert b_loc % BLK == 0
    n_blocks = b_loc // BLK

    nc = bacc.Bacc(
        "TRN2",
        target_bir_lowering=False,
        debug=False,
        enable_asserts=False,
        num_devices=num_devices,
    )

    featb = nc.dram_tensor("featb", [E, b_loc, D], bf16, kind="ExternalInput").ap()
    w1x = nc.dram_tensor("w1x", [D, E * CW], bf16, kind="ExternalInput").ap()
    qones = nc.dram_tensor("qones", [D, E * 8], bf16, kind="ExternalInput").ap()
    w2 = nc.dram_tensor("w2bf", [H, E], bf16, kind="ExternalInput").ap()
    outb = nc.dram_tensor("outb", [E, b_loc, D], bf16, kind="ExternalOutput").ap()
    if has_b1:
        b1row = nc.dram_tensor("b1row", [1, H], bf16, kind="ExternalInput").ap()
    if has_ln1:
        g_ln1 = nc.dram_tensor("g_ln1", [P, H], f32, kind="ExternalInput").ap()
        b_ln1 = nc.dram_tensor("b_ln1", [P, H], f32, kind="ExternalInput").ap()
    if has_b2:
        eb2 = nc.dram_tensor("eb2", [P, E], f32, kind="ExternalInput").ap()
    if has_outgb:
        g_out = nc.dram_tensor("g_out", [P, D], f32, kind="ExternalInput").ap()
        b_out = nc.dram_tensor("b_out", [P, D], f32, kind="ExternalInput").ap()

    feat_r = featb.rearrange("e (n p jj) d -> n p e jj d", p=P, jj=JJ)
    out_r = outb.rearrange("e (n p jj) d -> n p e jj d", p=P, jj=JJ)

    with tile.TileContext(nc) as tc, ExitStack() as ctx:
        # Chain every table-function ACT op in emission order so the Tile
        # scheduler cannot interleave ops from different act-function sets
        # (each set switch costs a ~1.3us LoadActFuncSet).
        _act_prev = [None]

        def act_ordered(inst):
            ins = inst.ins
            if _act_prev[0] is not None:
                tile.add_dep_helper(ins, _act_prev[0], sync=False,
                                    reason="act-table order")
            _act_prev[0] = ins
            return inst

        def act_load(set_id):
            # set 10 = gelu+helpers, set 6 = ln+exp+helpers
            return act_ordered(nc.scalar.add_instruction(
                mybir.InstLoadActFuncSet(
                    name=nc.get_next_instruction_name(), ins=[], outs=[],
                    act_func_set_id=set_id)))

        const_pool = ctx.enter_context(tc.tile_pool(name="const", bufs=1))
        ident_b = const_pool.tile([P, P], bf16)
        masks.make_identity(nc, ident_b[:])
        w1x_sb = const_pool.tile([D, E * CW], bf16)
        nc.sync.dma_start(w1x_sb[:], w1x)
        w1x3 = w1x_sb.rearrange("d (e c) -> d e c", e=E)
        qo_sb = const_pool.tile([D, E * 8], bf16)
        nc.sync.dma_start(qo_sb[:], qones)
        qo3 = qo_sb.rearrange("d (e c) -> d e c", e=E)
        w2_sb = const_pool.tile([H, E], bf16)
        nc.sync.dma_start(w2_sb[:], w2)
        hld = const_pool.tile([P, 1], f32)
        nc.vector.memset(hld[:], HALF_LN_D)
        epsc = const_pool.tile([P, 1], f32)
        nc.vector.memset(epsc[:], EPS)
        if has_b1:
            ones1 = const_pool.tile([1, P], bf16)
            nc.vector.memset(ones1[:], 1.0)
            b1_sb = const_pool.tile([1, H], bf16)
            nc.sync.dma_start(b1_sb[:], b1row)
        if has_ln1:
            gln_sb = const_pool.tile([P, H], f32)
            nc.sync.dma_start(gln_sb[:], g_ln1)
            bln_sb = const_pool.tile([P, H], f32)
            nc.sync.dma_start(bln_sb[:], b_ln1)
        if has_b2:
            eb2_sb = const_pool.tile([P, E], f32)
            nc.sync.dma_start(eb2_sb[:], eb2)
        if has_outgb:
            gout_sb = const_pool.tile([P, D], f32)
            nc.sync.dma_start(gout_sb[:], g_out)
            bout_sb = const_pool.tile([P, D], f32)
            nc.sync.dma_start(bout_sb[:], b_out)

        io_pool = ctx.enter_context(tc.tile_pool(name="io", bufs=3))
        xt_pool = ctx.enter_context(tc.tile_pool(name="xt", bufs=2))
        xq_pool = ctx.enter_context(tc.tile_pool(name="xq", bufs=3))
        hb_pool = ctx.enter_context(tc.tile_pool(name="hb", bufs=2))
        sm_pool = ctx.enter_context(tc.tile_pool(name="sm", bufs=3))
        st_pool = ctx.enter_context(tc.tile_pool(name="st", bufs=2))
        ps_t = ctx.enter_context(tc.tile_pool(name="ps_t", bufs=2, space="PSUM"))
        ps_g = ctx.enter_context(tc.tile_pool(name="ps_g", bufs=2, space="PSUM"))
        ps_s = ctx.enter_context(tc.tile_pool(name="ps_s", bufs=2, space="PSUM"))
        ps_h = ctx.enter_context(tc.tile_pool(name="ps_h", bufs=2, space="PSUM"))

        SQ_BATCH = 4   # jj sub-tiles per gpsimd square op
        Q_DEFER = 2    # defer q-matmuls by this many square groups
        LEAD = 1       # transposes run this many jj ahead of gate matmuls
        PRE = 3        # transposes of block n+1 emitted before tail of n

        NG = JJ // SQ_BATCH
        state = {}

        def gen_p1(n):
            """Transposes, copies, squares, gate matmuls, q matmuls, gelu,
            LN1 stats for block n.  Yields once early so a few transposes
            land in the PE queue ahead of the previous block's tail."""
            x = io_pool.tile([P, E * JJ * D], bf16, tag="x", name=f"x_{n}")
            x4 = x.rearrange("p (e jj d) -> p e jj d", e=E, jj=JJ)
            nc.sync.dma_start(x4, feat_r[n])

            sS = st_pool.tile([P, JJ * E], f32, tag="sS", name=f"sS_{n}")
            sS3 = sS.rearrange("p (jj e) -> p jj e", jj=JJ)
            ln = st_pool.tile([P, JJ * 2], f32, tag="ln", name=f"ln_{n}")
            ln3 = ln.rearrange("p (jj s) -> p jj s", jj=JJ)
            hg = hb_pool.tile([P, JJ * H], bf16, tag="hg", name=f"hg_{n}")
            hg3 = hg.rearrange("p (jj h) -> p jj h", jj=JJ)
            psS = ps_s.tile([P, 512], f32, tag="psS", name=f"psS_{n}")
            state[n] = (x, x4, sS, sS3, ln, ln3, hg, hg3, psS)

            xtgs = []
            xqgs = []

            def emit_transp(jj):
                psT = ps_t.tile([P, E * D], bf16, tag="psT",
                                name=f"psT_{n}_{jj}")
                psT3 = psT.rearrange("p (e b) -> p e b", e=E)
                for e in range(E):
                    nc.tensor.matmul(psT3[:, e], x4[:, e, jj], ident_b[:],
                                     is_transpose=True)
                if jj % SQ_BATCH == 0:
                    xtgs.append(xt_pool.tile(
                        [P, SQ_BATCH * E * D], bf16, tag="xtg",
                        name=f"xtg_{n}_{jj // SQ_BATCH}"))
                    xqgs.append(xq_pool.tile(
                        [P, SQ_BATCH * E * D], bf16, tag="xqg",
                        name=f"xqg_{n}_{jj // SQ_BATCH}"))
                k = jj % SQ_BATCH
                xtg = xtgs[-1]
                dst = xtg.rearrange("p (k r) -> p k r", k=SQ_BATCH)[:, k]
                if jj % 2 == 0:
                    nc.scalar.activation(dst, psT[:], AF.Copy)
                else:
                    nc.vector.tensor_copy(dst, psT[:])
                if k == SQ_BATCH - 1:
                    nc.gpsimd.tensor_mul(xqgs[-1][:], xtg[:], xtg[:])

            def emit_gate(jj):
                g = jj // SQ_BATCH
                k = jj % SQ_BATCH
                xt3 = xtgs[g].rearrange("p (k e b) -> p k e b",
                                        k=SQ_BATCH, e=E)[:, k]
                psG = ps_g.tile([P, 512], f32, tag="psG", name=f"psG_{n}_{jj}")
                for e in range(E):
                    nc.tensor.matmul(
                        psG[:, 0:CW], xt3[:, e], w1x3[:, e],
                        start=(e == 0), stop=(e == E - 1 and not has_b1),
                    )
                if has_b1:
                    nc.tensor.matmul(psG[:, 0:H], ones1[:], b1_sb[:],
                                     start=False, stop=True)
                gelu_fn = AF.Tanh if sim_tanh else AF.Gelu
                act_ordered(nc.scalar.activation(hg3[:, jj], psG[:, 0:H],
                                                 gelu_fn, bias=0.0, scale=1.0))
                s1 = sm_pool.tile([P, 6], f32, tag="s1",
                                  name=f"s1_{n}_{jj}")
                nc.vector.bn_stats(s1[:], hg3[:, jj])
                nc.vector.bn_aggr(ln3[:, jj], s1[:])
                nc.vector.tensor_copy(sS3[:, jj], psG[:, H:H + E])

            def emit_q(g):
                xq4 = xqgs[g].rearrange("p (k e b) -> p k e b",
                                        k=SQ_BATCH, e=E)
                for k in range(SQ_BATCH):
                    jq = g * SQ_BATCH + k
                    for e in range(E):
                        nc.tensor.matmul(
                            psS[:, jq * E:(jq + 1) * E], xq4[:, k, e],
                            qo3[:, e],
                            start=(e == 0), stop=(e == E - 1),
                        )

            for jj in range(PRE):
                emit_transp(jj)
            yield

            act_load(10)
            for j in range(JJ):
                if j + PRE < JJ:
                    emit_transp(j + PRE)
                emit_gate(j)
                g = j // SQ_BATCH
                if j % SQ_BATCH == SQ_BATCH - 1 and g >= Q_DEFER:
                    emit_q(g - Q_DEFER)
            for g in range(NG - Q_DEFER, NG):
                emit_q(g)

        def tail_head(n):
            """LN1 scalar math, LN1 apply, logits, exp, final-LN math."""
            x, x4, sS, sS3, ln, ln3, hg, hg3, psS = state[n]
            hl = hb_pool.tile([P, JJ * H], bf16, tag="hl", name=f"hl_{n}")
            hl3 = hl.rearrange("p (jj h) -> p jj h", jj=JJ)
            zz = st_pool.tile([P, JJ * E], f32, tag="zz", name=f"zz_{n}")
            zz3 = zz.rearrange("p (jj e) -> p jj e", jj=JJ)
            zs = st_pool.tile([P, JJ], f32, tag="zs", name=f"zs_{n}")

            act_load(6)
            lnv = st_pool.tile([P, JJ], f32, tag="lnv", name=f"lnv_{n}")
            act_ordered(nc.scalar.activation(lnv[:], ln3[:, :, 1], AF.Ln,
                                             bias=epsc[:], scale=1.0))
            rs1 = st_pool.tile([P, JJ], f32, tag="rs1", name=f"rs1_{n}")
            act_ordered(nc.scalar.activation(rs1[:], lnv[:], AF.Exp,
                                             bias=0.0, scale=-0.5))

            for jj in range(JJ):
                nc.vector.tensor_scalar(
                    hl3[:, jj], hg3[:, jj], ln3[:, jj, 0:1],
                    rs1[:, jj:jj + 1], AO.subtract, AO.mult,
                )
                if has_ln1:
                    nc.vector.tensor_mul(hl3[:, jj], hl3[:, jj], gln_sb[:])
                    nc.vector.tensor_add(hl3[:, jj], hl3[:, jj], bln_sb[:])

            # batched hln transposes -> 2 big copies -> logits -> exps
            hlts = []
            for half in range(2):
                psH = ps_h.tile([P, 8 * H], bf16, tag="psH",
                                name=f"psH_{n}_{half}")
                psH3 = psH.rearrange("p (k b) -> p k b", k=8)
                for k in range(8):
                    nc.tensor.matmul(psH3[:, k], hl3[:, half * 8 + k],
                                     ident_b[:], is_transpose=True)
                hlt = sm_pool.tile([P, 8 * H], bf16, tag="hlt",
                                   name=f"hlt_{n}_{half}")
                nc.scalar.activation(hlt[:], psH[:], AF.Copy)
                hlts.append(hlt.rearrange("p (k b) -> p k b", k=8))
            for jj in range(JJ):
                nc.tensor.matmul(psS[:, 256 + jj * E:256 + (jj + 1) * E],
                                 hlts[jj // 8][:, jj % 8], w2_sb[:],
                                 start=True, stop=True)
            for jj in range(JJ):
                act_ordered(nc.scalar.activation(
                    zz3[:, jj], psS[:, 256 + jj * E:256 + (jj + 1) * E],
                    AF.Exp, bias=0.0, scale=1.0))
                if has_b2:
                    nc.vector.tensor_mul(zz3[:, jj], zz3[:, jj], eb2_sb[:])

            # batched final-LN scalar math
            nc.vector.reduce_sum(zs[:], zz3, axis=mybir.AxisListType.X)
            sQ = st_pool.tile([P, JJ * E], f32, tag="sQ", name=f"sQ_{n}")
            nc.vector.tensor_copy(sQ[:], psS[:, 0:JJ * E])
            msq = st_pool.tile([P, JJ * E], f32, tag="msq", name=f"msq_{n}")
            nc.vector.scalar_tensor_tensor(msq[:], sS[:], 1.0 / D, sS[:],
                                           AO.mult, AO.mult)
            m2 = st_pool.tile([P, JJ * E], f32, tag="m2", name=f"m2_{n}")
            nc.vector.tensor_sub(m2[:], sQ[:], msq[:])
            zz2 = st_pool.tile([P, JJ * E], f32, tag="zz2", name=f"zz2_{n}")
            nc.vector.tensor_mul(zz2[:], zz[:], zz[:])
            u = st_pool.tile([P, JJ * E], f32, tag="u", name=f"u_{n}")
            nc.vector.tensor_mul(u[:], zz2[:], m2[:])
            zeps = st_pool.tile([P, JJ], f32, tag="zeps", name=f"zeps_{n}")
            nc.vector.scalar_tensor_tensor(zeps[:], zs[:], float(D) * EPS,
                                           zs[:], AO.mult, AO.mult)
            u2 = st_pool.tile([P, JJ * E], f32, tag="u2", name=f"u2_{n}")
            zb = zeps.rearrange("p (jj o) -> p jj o", o=1).broadcast_to(
                (P, JJ, E))
            nc.vector.tensor_add(u2.rearrange("p (jj e) -> p jj e", jj=JJ),
                                 u.rearrange("p (jj e) -> p jj e", jj=JJ), zb)
            l2 = st_pool.tile([P, JJ * E], f32, tag="l2", name=f"l2_{n}")
            act_ordered(nc.scalar.activation(l2[:], u2[:], AF.Ln,
                                             bias=0.0, scale=1.0))
            qq = st_pool.tile([P, JJ * E], f32, tag="qq", name=f"qq_{n}")
            act_ordered(nc.scalar.activation(qq[:], l2[:], AF.Exp,
                                             bias=hld[:], scale=-0.5))
            aa = st_pool.tile([P, JJ * E], f32, tag="aa", name=f"aa_{n}")
            nc.vector.tensor_mul(aa[:], zz[:], qq[:])
            bn = st_pool.tile([P, JJ * E], f32, tag="bn", name=f"bn_{n}")
            nc.vector.scalar_tensor_tensor(bn[:], sS[:], -1.0 / D, aa[:],
                                           AO.mult, AO.mult)
            state[n] = (x, x4, aa, bn)

        def tail_apply(n):
            """Final applies (in place) + store.  Copy/Identity only, so
            these float freely in the ACT queue (no table dependency)."""
            x, x4, aa, bn = state.pop(n)
            for jj in range(JJ):
                for e in range(E):
                    c = jj * E + e
                    if c >= 60:
                        nc.scalar.activation(
                            x4[:, e, jj], x4[:, e, jj], AF.Identity,
                            bias=bn[:, c:c + 1], scale=aa[:, c:c + 1])
                    else:
                        nc.vector.tensor_scalar(
                            x4[:, e, jj], x4[:, e, jj],
                            aa[:, c:c + 1], bn[:, c:c + 1], AO.mult, AO.add)
                    if has_outgb:
                        nc.vector.tensor_mul(x4[:, e, jj], x4[:, e, jj],
                                             gout_sb[:])
                        nc.vector.tensor_add(x4[:, e, jj], x4[:, e, jj],
                                             bout_sb[:])
            nc.scalar.dma_start(out_r[n], x4)

        # Software pipeline: while the tail of block n runs on DVE/ACT,
        # the PE grinds through block n+1's transposes and gate matmuls.
        gens = [gen_p1(n) for n in range(n_blocks)]
        for _ in gens[0]:
            pass
        for n in range(n_blocks):
            if n + 1 < n_blocks:
                next(gens[n + 1])       # DMA + first PRE transposes
            tail_head(n)
            if n + 1 < n_blocks:
                for _ in gens[n + 1]:   # rest of block n+1 phase 1
                    pass
            tail_apply(n)

    nc.compile()
    return nc


def _get_nc(b_loc, flags, num_devices):
    key = (b_loc, flags, num_devices)
    if key not in _NC_CACHE:
        _NC_CACHE[key] = _build_nc(b_loc, *flags, num_devices=num_devices)
    return _NC_CACHE[key]


def _host_inputs(gate_w1, gate_b1, ln1_g, ln1_b, gate_w2, gate_b2, out_g, out_b,
                 flags):
    import ml_dtypes
    bf = ml_dtypes.bfloat16
    has_b1, has_ln1, has_b2, has_outgb = flags

    w1r = gate_w1.reshape(E, D, H)
    w1x = np.zeros((D, E, CW), dtype=bf)
    w1x[:, :, 0:H] = w1r.transpose(1, 0, 2).astype(bf)
    for e in range(E):
        w1x[:, e, H + e] = bf(1.0)
    qones = np.zeros((D, E, 8), dtype=bf)
    for e in range(E):
        qones[:, e, e] = bf(1.0)

    common = {
        "w1x": np.ascontiguousarray(w1x.reshape(D, E * CW)),
        "qones": np.ascontiguousarray(qones.reshape(D, E * 8)),
        "w2bf": np.ascontiguousarray(gate_w2.astype(bf)),
    }
    if has_b1:
        common["b1row"] = np.ascontiguousarray(gate_b1.reshape(1, H).astype(bf))
    if has_ln1:
        common["g_ln1"] = np.ascontiguousarray(np.tile(ln1_g, (P, 1)))
        common["b_ln1"] = np.ascontiguousarray(np.tile(ln1_b, (P, 1)))
    if has_b2:
        common["eb2"] = np.ascontiguousarray(
            np.tile(np.exp(gate_b2.astype(np.float64)).astype(np.float32),
                    (P, 1)))
    if has_outgb:
        common["g_out"] = np.ascontiguousarray(np.tile(out_g, (P, 1)))
        common["b_out"] = np.ascontiguousarray(np.tile(out_b, (P, 1)))
    return common


def kernel(**inputs):
    import ml_dtypes
    from concourse.bass_utils import run_bass_kernel_spmd

    features = np.asarray(inputs["features"], dtype=np.float32)
    gate_w1 = np.asarray(inputs["gate_w1"], dtype=np.float32)
    gate_b1 = np.asarray(inputs["gate_b1"], dtype=np.float32)
    ln1_g = np.asarray(inputs["ln1_g"], dtype=np.float32)
    ln1_b = np.asarray(inputs["ln1_b"], dtype=np.float32)
    gate_w2 = np.asarray(inputs["gate_w2"], dtype=np.float32)
    gate_b2 = np.asarray(inputs["gate_b2"], dtype=np.float32)
    out_g = np.asarray(inputs["out_g"], dtype=np.float32)
    out_b = np.asarray(inputs["out_b"], dtype=np.float32)

    e, B, d = features.shape
    assert e == E and d == D
    assert B % (N_CORES * BLK) == 0
    b_loc = B // N_CORES

    has_b1 = bool(np.any(gate_b1 != 0))
    has_ln1 = bool(np.any(ln1_g != 1) or np.any(ln1_b != 0))
    has_b2 = bool(np.any(gate_b2 != 0))
    has_outgb = bool(np.any(out_g != 1) or np.any(out_b != 0))
    flags = (has_b1, has_ln1, has_b2, has_outgb)

    nc = _get_nc(b_loc, flags, num_devices=1)

    bf = ml_dtypes.bfloat16
    common = _host_inputs(gate_w1, gate_b1, ln1_g, ln1_b, gate_w2, gate_b2,
                          out_g, out_b, flags)
    featb = features.astype(bf)

    in_maps = []
    for c in range(N_CORES):
        m = dict(common)
        m["featb"] = np.ascontiguousarray(featb[:, c * b_loc:(c + 1) * b_loc, :])
        in_maps.append(m)

    res = run_bass_kernel_spmd(nc, in_maps, core_ids=list(range(N_CORES)))
    global LAST_RESULTS
    LAST_RESULTS = res
    out = np.concatenate([r["outb"] for r in res.results], axis=1)
    return np.ascontiguousarray(out.astype(np.float32))


LAST_RESULTS = None
